# revision 76
# baseline (speedup 1.0000x reference)
"""BiasedMultiHeadAttention Trainium2 kernel (restructured).

Sharding: 8 cores = (batch b, query-half qh); per-core x rows host-rolled so
the query block is rows 0..511 -> one SPMD program for all cores.

Structure vs baseline:
- x is host-transposed (xtc = x^T); LayerNorm applied as a column affine
  (xn^T = x^T*r_row - (mu*r)_row) using PE-transposed stats rows broadcast
  across partitions.  No on-device transposes of x, no serial LN->transpose
  startup.
- Projections and attention interleaved per head-pair t: emit attn(t-1)
  then proj(t).  K/Q tiles are per-pair pool tiles (precise deps).
- Softmax bias-add split across DVE (head A) and Pool (head B).
- Normalization (rowsum reciprocal) has no DRAM roundtrip
  (gpsimd.partition_broadcast) and is software-pipelined into the next
  pair's chunk loop.
- Optional fp8 DoubleRow paths (Q/K projections, V/AV, out-proj) and fp8
  bias DMA.
"""

import numpy as np
import ml_dtypes

import concourse.bass as bass
import concourse.tile as tile
import concourse.mybir as mybir
from concourse import bacc
from concourse.bass_utils import run_bass_kernel_spmd

B, L, E, H = 4, 1024, 1024, 16
D = E // H
SCALE = D**-0.5
EPS = 1e-5
NCORES = 8
QL = 512
PT = 128
NL = L // PT
NE = E // PT
HP = H // 2

F32 = mybir.dt.float32
BF16 = mybir.dt.bfloat16
FP8 = mybir.dt.float8e4
BF_NP = ml_dtypes.bfloat16
FP8_NP = ml_dtypes.float8_e4m3

# ---- feature flags ----
FP8_QKPROJ = True    # Q/K projections in fp8 DoubleRow
FP8_V = True         # V projection in fp8 DoubleRow
FP8_AV = True        # attn weights + V in fp8, DoubleRow AV matmuls
FP8_BIAS = True      # bias tensor DMA'd as fp8
FP8_OUT = True       # out-projection in fp8 DoubleRow
WSCALE = 32.0        # host premultiplier for fp8 weights (sigma -> ~1)
SHIFT = -1.0         # exp shift (fp8 attn-weight range centering)
MM_ADD = (0, 1, 2, 3, 4, 5, 6, 7)  # chunks whose bias add is a PE identity-matmul
NORM_AT = 1          # chunk index inside pair t at which norm(t-1) is emitted

LAST_RESULT = None


def _build_nc(gates, use_pbias, use_mask, dump=()):
    nc = bacc.Bacc("TRN2", target_bir_lowering=False, debug=False)
    same_gate = len(set(gates)) == 1

    def dump_tile(name, ap):
        if name in dump:
            d = nc.dram_tensor("d_" + name, list(ap.shape), ap.dtype,
                               kind="ExternalOutput")
            nc.sync.dma_start(d[tuple(slice(None) for _ in ap.shape)], ap)
    PRJ8 = FP8_QKPROJ
    W8 = mybir.MatmulPerfMode.DoubleRow

    xcb_d = nc.dram_tensor("xcb", [PT, NL, L], BF16, kind="ExternalInput")
    xt_d = nc.dram_tensor("xtc", [PT, NE, L], BF16, kind="ExternalInput")
    xres_d = nc.dram_tensor("xres", [PT, 4, L], F32, kind="ExternalInput")
    bias_d = nc.dram_tensor("biasc", [L, H, QL], FP8 if FP8_BIAS else BF16,
                            kind="ExternalInput")
    wq_d = nc.dram_tensor("wqt", [PT, NE, E], FP8 if PRJ8 else BF16,
                          kind="ExternalInput")
    wk_d = nc.dram_tensor("wkt", [PT, NE, E], FP8 if PRJ8 else BF16,
                          kind="ExternalInput")
    wv_d = nc.dram_tensor("wvt", [PT, NE, E], FP8 if FP8_V else BF16,
                          kind="ExternalInput")
    wo_d = nc.dram_tensor("wot", [PT, NE, E], FP8 if FP8_OUT else BF16,
                          kind="ExternalInput")
    pb_d = {}
    for name, use in zip("qkvo", use_pbias):
        if use:
            pb_d[name] = nc.dram_tensor(f"b{name}e", [1, E], BF16,
                                        kind="ExternalInput")
    if use_mask:
        km_d = nc.dram_tensor("kmc", [PT, NL], F32, kind="ExternalInput")
        mq_d = nc.dram_tensor("mqc", [1, QL], F32, kind="ExternalInput")
    y_d = nc.dram_tensor("yc", [QL, E], F32, kind="ExternalOutput")

    AT_DT = FP8 if FP8_AV else BF16
    OT_DT = FP8 if FP8_OUT else BF16
    V_DT = FP8 if FP8_AV else BF16
    # descale factors applied when copying projection psums to SBUF
    qk_dsc = (1.0 / WSCALE) if PRJ8 else None
    v_dsc = (1.0 / WSCALE) if FP8_V else None

    with tile.TileContext(nc) as tc:
        with (
            tc.tile_pool(name="persist", bufs=1) as pp,
            tc.tile_pool(name="consts", bufs=1) as cp,
        ):
            ident = cp.tile([PT, PT], BF16)
            nc.gpsimd.memset(ident, 0.0)
            nc.gpsimd.affine_select(
                out=ident, in_=ident,
                compare_op=mybir.AluOpType.not_equal, fill=1.0,
                base=0, pattern=[[-1, PT]], channel_multiplier=1)
            identb = ident
            if FP8_BIAS:
                identb = cp.tile([PT, PT], FP8)
                nc.gpsimd.tensor_copy(identb, ident)
            eps_t = cp.tile([PT, 1], F32)
            nc.vector.memset(eps_t, EPS)
            dummy = cp.tile([1, 2], F32)
            nc.vector.memset(dummy, 0.0)
            shift_t = None
            if FP8_AV:
                shift_t = cp.tile([PT, 1], F32)
                nc.vector.memset(shift_t, SHIFT)
            ones_row = None
            if pb_d:
                ones_row = cp.tile([1, QL], BF16)
                nc.vector.memset(ones_row, 1.0)
            if use_mask:
                km_sb = cp.tile([PT, NL], F32)
                nc.sync.dma_start(km_sb, km_d[:, :])
                mqrow = cp.tile([1, QL], F32)
                nc.sync.dma_start(mqrow, mq_d[:, :])
                mqb = cp.tile([64, QL], F32)
                nc.gpsimd.partition_broadcast(mqb, mqrow[0:1, :])

            # ---- persistent tensors ----
            # DMA priority: x chunks first (stats are the critical path),
            # then x^T, then weights in first-use order; wo last.
            xt_sb = pp.tile([PT, NE, L], BF16)
            wk_sb = pp.tile([PT, NE, E], FP8 if PRJ8 else BF16)
            wq_sb = pp.tile([PT, NE, E], FP8 if PRJ8 else BF16)
            wv_sb = pp.tile([PT, NE, E], FP8 if FP8_V else BF16)
            wo_sb = pp.tile([PT, NE, E], FP8 if FP8_OUT else BF16)
            pbr = {}
            for name in pb_d:
                pbr[name] = cp.tile([1, E], BF16)
                nc.sync.dma_start(pbr[name], pb_d[name][:, :])

            xnTb = pp.tile([PT, NE, L], BF16)   # normalized x^T
            xnT8 = None
            if PRJ8 or FP8_V:
                xnT8 = pp.tile([PT, NE, L], FP8)
            # V | ones col (| zero pad to 128 when DoubleRow AV: LDWEIGHTS
            # in DR mode requires 128-wide weight blocks)
            VW = 128 if FP8_AV else 65
            v3 = pp.tile([PT, NL, H, VW], V_DT)
            oT = pp.tile([PT, NE, QL], OT_DT)    # normalized attnout^T
            murb = pp.tile([PT, L], BF16)        # broadcast mu*r row
            rsrb = pp.tile([PT, L], BF16)        # broadcast r row

            # ========== Phase 0: stats ==========
            with (
                tc.tile_pool(name="st", bufs=2) as sp,
                tc.tile_pool(name="stp", bufs=1, space="PSUM") as stp,
            ):
                xcb_sb = sp.tile([PT, NL, L], BF16, tag="xcb", bufs=1)
                for lt in range(NL):
                    nc.sync.dma_start(xcb_sb[:, lt, :], xcb_d[:, lt, :])
                for ec in range(NE):
                    nc.sync.dma_start(xt_sb[:, ec, :], xt_d[:, ec, :])
                for ec in range(NE):
                    nc.sync.dma_start(wk_sb[:, ec, :], wk_d[:, ec, :])
                mvall = sp.tile([PT, NL, 2], F32, tag="mv")
                for lt in range(NL):
                    xr = xcb_sb[:, lt, :].rearrange("p (s d) -> p s d", s=2)
                    stats = sp.tile([PT, 2, 6], F32, tag="stats", bufs=3)
                    for sg in range(2):
                        nc.vector.bn_stats(stats[:, sg, :], xr[:, sg, :])
                    nc.vector.bn_aggr(mvall[:, lt, :], stats)
                sd = sp.tile([PT, NL], F32, tag="sd")
                nc.scalar.activation(sd, mvall[:, :, 1],
                                     mybir.ActivationFunctionType.Sqrt,
                                     bias=eps_t)
                # re-prime EXP after the sqrt so attention EXPs don't pay a
                # table reload
                nc.scalar.activation(dummy, sd[0:1, 0:2],
                                     mybir.ActivationFunctionType.Exp)
                rs = sp.tile([PT, NL], F32, tag="rs")
                nc.vector.reciprocal(rs, sd)
                pr2 = sp.tile([PT, 2, NL], BF16, tag="pr2")
                nc.vector.tensor_copy(pr2[:, 0, :], rs)
                nc.vector.tensor_mul(pr2[:, 1, :], mvall[:, :, 0], rs)
                psT = stp.tile([16, PT], BF16, tag="stT")
                nc.tensor.transpose(psT, pr2.rearrange("p a b -> p (a b)"),
                                    ident)
                prT = sp.tile([16, PT], BF16, tag="prT")
                nc.vector.tensor_copy(prT, psT)
                rsrow = sp.tile([1, L], BF16, tag="rsrow")
                nc.sync.dma_start(rsrow, prT[0:8, :])
                murow = sp.tile([1, L], BF16, tag="murow")
                nc.sync.dma_start(murow, prT[8:16, :])
                # remaining weights after the tiny row DMAs
                for ec in range(NE):
                    nc.sync.dma_start(wq_sb[:, ec, :], wq_d[:, ec, :])
                for ec in range(NE):
                    nc.sync.dma_start(wv_sb[:, ec, :], wv_d[:, ec, :])
                for ec in range(NE):
                    nc.sync.dma_start(wo_sb[:, ec, :], wo_d[:, ec, :])
                nc.gpsimd.partition_broadcast(rsrb, rsrow[0:1, :])
                nc.gpsimd.partition_broadcast(murb, murow[0:1, :])
                # v3 ones/zero-pad memsets after the broadcasts on Pool
                nc.gpsimd.memset(v3[:, :, :, 64:65], 1.0)
                if FP8_AV:
                    for lt in range(NL):
                        nc.gpsimd.memset(v3[:, lt, :, 65:128], 0.0)
                for lh in range(2):
                    lsl = slice(lh * QL, (lh + 1) * QL)
                    for ec in range(NE):
                        nc.vector.tensor_mul(xnTb[:, ec, lsl],
                                             xt_sb[:, ec, lsl], rsrb[:, lsl])
                        if PRJ8 and FP8_V:
                            nc.vector.tensor_sub(xnT8[:, ec, lsl],
                                                 xnTb[:, ec, lsl],
                                                 murb[:, lsl])
                        else:
                            nc.vector.tensor_sub(xnTb[:, ec, lsl],
                                                 xnTb[:, ec, lsl],
                                                 murb[:, lsl])
                            if xnT8 is not None:
                                nc.vector.tensor_copy(xnT8[:, ec, lsl],
                                                      xnTb[:, ec, lsl])
                dump_tile("rsrb", rsrb[:, :])
                dump_tile("murb", murb[:, :])
                dump_tile("xnTb", xnTb[:, :, :])

            # ========== interleaved projections + attention ==========
            with (
                tc.tile_pool(name="kq", bufs=1) as kqp,
                tc.tile_pool(name="ps", bufs=2, space="PSUM") as psp,
                tc.tile_pool(name="av", bufs=2, space="PSUM") as avp,
                tc.tile_pool(name="bias", bufs=8) as bp,
                tc.tile_pool(name="s1p", bufs=3) as s1p,
                tc.tile_pool(name="attn", bufs=3) as ap,
                tc.tile_pool(name="rbs", bufs=2) as rp,
                tc.tile_pool(name="oo", bufs=2) as oop,
                tc.tile_pool(name="yo", bufs=2) as yop,
            ):
                KBUFS = 3
                kq_tiles = {}   # t -> (kA, kB, qT)
                av_tiles = {}   # t -> (avA, avB)

                def qk_mm(ps, w, osl, nsl, extra=None, x8=False):
                    """accumulate ps += w[:, :, osl]^T @ xnT[:, :, nsl]"""
                    xn = xnT8 if x8 else xnTb
                    if x8:
                        for j in range(NE // 2):
                            nc.tensor.matmul(
                                ps, w[:, 2 * j:2 * j + 2, osl],
                                xn[:, 2 * j:2 * j + 2, nsl],
                                start=(j == 0),
                                stop=(j == NE // 2 - 1 and extra is None),
                                perf_mode=W8)
                    else:
                        for j in range(NE):
                            nc.tensor.matmul(
                                ps, w[:, j, osl], xn[:, j, nsl],
                                start=(j == 0),
                                stop=(j == NE - 1 and extra is None))
                    if extra is not None:
                        nc.tensor.matmul(ps, extra[:, osl],
                                         ones_row[:, 0:nsl.stop - nsl.start],
                                         start=False, stop=True)

                def emit_proj(t):
                    osl = slice(t * PT, (t + 1) * PT)
                    kA = kqp.tile([PT, L], BF16, tag="kA", bufs=KBUFS)
                    kB = kqp.tile([PT, L], BF16, tag="kB", bufs=KBUFS)
                    qT = kqp.tile([PT, QL], BF16, tag="qT", bufs=KBUFS)
                    if t < KBUFS:
                        nc.gpsimd.memset(kA[64:PT, :], 0.0)
                        nc.gpsimd.memset(kB[0:64, :], 0.0)
                    for nh in range(2):
                        nsl = slice(nh * QL, (nh + 1) * QL)
                        psK = psp.tile([PT, QL], F32, tag=f"ps{nh}")
                        qk_mm(psK, wk_sb, osl, nsl,
                              extra=pbr.get("k"), x8=PRJ8)
                        if qk_dsc is None:
                            nc.vector.tensor_copy(kA[0:64, nsl],
                                                  psK[0:64, :])
                            nc.vector.tensor_copy(kB[64:PT, nsl],
                                                  psK[64:PT, :])
                        else:
                            nc.vector.tensor_scalar_mul(
                                kA[0:64, nsl], psK[0:64, :], qk_dsc)
                            nc.vector.tensor_scalar_mul(
                                kB[64:PT, nsl], psK[64:PT, :], qk_dsc)
                    psQ = psp.tile([PT, QL], F32, tag="ps0")
                    qk_mm(psQ, wq_sb, osl, slice(0, QL),
                          extra=pbr.get("q"), x8=PRJ8)
                    if qk_dsc is None:
                        nc.scalar.copy(qT, psQ)
                    else:
                        nc.scalar.activation(
                            qT, psQ,
                            mybir.ActivationFunctionType.Copy,
                            scale=qk_dsc)
                    kq_tiles[t] = (kA, kB, qT)
                    if t == 0:
                        dump_tile("kA0", kA[:, :])
                        dump_tile("kB0", kB[:, :])
                        dump_tile("qT0", qT[:, :])
                    # V projection: emit half of V in t=0, other half in t=1
                    if t < 2:
                        vsl = slice(t * QL, (t + 1) * QL)
                        for lt in range(NL):
                            lsl = slice(lt * PT, (lt + 1) * PT)
                            psV = psp.tile([PT, QL], F32, tag="ps1",
                                           name="psV")
                            xn = xnT8 if FP8_V else xnTb
                            if FP8_V:
                                for j in range(NE // 2):
                                    nc.tensor.matmul(
                                        psV, xn[:, 2 * j:2 * j + 2, lsl],
                                        wv_sb[:, 2 * j:2 * j + 2, vsl],
                                        start=(j == 0),
                                        stop=(j == NE // 2 - 1
                                              and "v" not in pbr),
                                        perf_mode=W8)
                            else:
                                for j in range(NE):
                                    nc.tensor.matmul(
                                        psV, xn[:, j, lsl],
                                        wv_sb[:, j, vsl],
                                        start=(j == 0),
                                        stop=(j == NE - 1
                                              and "v" not in pbr))
                            if "v" in pbr:
                                nc.tensor.matmul(psV, ones_row[:, 0:PT],
                                                 pbr["v"][:, vsl],
                                                 start=False, stop=True)
                            dst = v3[:, lt, t * 8:(t + 1) * 8, 0:64]
                            src = psV.rearrange("p (h d) -> p h d", h=8)
                            eng = nc.vector if lt % 2 == 0 else nc.scalar
                            if v_dsc is None:
                                eng.tensor_copy(dst, src) \
                                    if eng is nc.vector else eng.copy(dst, src)
                            else:
                                if eng is nc.vector:
                                    eng.tensor_scalar_mul(dst, src, v_dsc)
                                else:
                                    eng.activation(
                                        dst, src,
                                        mybir.ActivationFunctionType.Copy,
                                        scale=v_dsc)

                def emit_norm(p):
                    avA, avB = av_tiles.pop(p)
                    osc = float(WSCALE) if FP8_OUT else 1.0
                    if p == 0 and "avA0" in dump:
                        avd = rp.tile([VW, QL], F32, tag="avd", bufs=1)
                        nc.vector.tensor_copy(avd, avA)
                        dump_tile("avA0", avd[:, :])
                    for hi, av in enumerate((avA, avB)):
                        rr = rp.tile([65, QL], F32, tag=f"rr{hi}", bufs=1)
                        nc.vector.tensor_copy(rr[64:65, :], av[64:65, :])
                        rr0 = rp.tile([1, QL], F32, tag=f"rr0{hi}", bufs=1)
                        nc.sync.dma_start(rr0, rr[64:65, :])
                        rsb = rp.tile([64, QL], F32, tag=f"rsb{hi}", bufs=1)
                        nc.gpsimd.partition_broadcast(rsb, rr0[0:1, :])
                        nc.vector.reciprocal_approx_fast(out=rsb, in_=rsb)
                        if use_mask:
                            nc.vector.tensor_mul(rsb, rsb, mqb)
                        if hi == 0:
                            nc.vector.scalar_tensor_tensor(
                                oT[0:64, p, :], av[0:64, :], osc, rsb,
                                op0=mybir.AluOpType.mult,
                                op1=mybir.AluOpType.mult)
                        else:
                            oto = oop.tile([64, QL], OT_DT, tag="oo")
                            nc.vector.scalar_tensor_tensor(
                                oto, av[0:64, :], osc, rsb,
                                op0=mybir.AluOpType.mult,
                                op1=mybir.AluOpType.mult)
                            nc.sync.dma_start(oT[64:PT, p, :], oto)

                def emit_attn(t, mid_fn=None):
                    kA, kB, qT = kq_tiles.pop(t)
                    hA, hB = 2 * t, 2 * t + 1
                    avA = avp.tile([VW, QL], F32, tag="avA")
                    avB = avp.tile([VW, QL], F32, tag="avB")
                    av_tiles[t] = (avA, avB)
                    at2 = None
                    for c in range(NL):
                        csl = slice(c * PT, (c + 1) * PT)
                        mm_add = c in MM_ADD
                        bt = bp.tile([PT, 2, QL], FP8 if FP8_BIAS else BF16,
                                     tag="bt")
                        nc.sync.dma_start(bt, bias_d[csl, hA:hB + 1, :])
                        if use_mask:
                            kmb = km_sb[:, c:c + 1]
                        elif FP8_AV:
                            kmb = shift_t
                        else:
                            kmb = 0.0
                        ci = c % 2
                        if ci == 0:
                            at2 = ap.tile([PT, 2, 2, QL], AT_DT, tag="at",
                                          bufs=4)
                        # separate per-head psum tiles + per-head EXPs keep
                        # the A-half pipeline independent of the B-half
                        for hi, kT in enumerate((kA, kB)):
                            psh = psp.tile([PT, QL], F32, tag=f"ps{hi}",
                                           name="psh")
                            nc.tensor.matmul(psh, kT[:, csl], qT,
                                             start=True, stop=not mm_add)
                            if mm_add:
                                nc.tensor.matmul(psh, identb, bt[:, hi, :],
                                                 start=False, stop=True)
                                s_in = psh
                            else:
                                s1 = s1p.tile([PT, QL], BF16, tag=f"s1{hi}")
                                nc.vector.tensor_add(s1, psh, bt[:, hi, :])
                                s_in = s1
                            nc.scalar.activation(
                                at2[:, ci, hi, :], s_in,
                                mybir.ActivationFunctionType.Exp,
                                bias=kmb, scale=gates[hA + hi])
                        if FP8_AV:
                            if ci == 1:
                                cp2 = c // 2
                                for hi, av in enumerate((avA, avB)):
                                    nc.tensor.matmul(
                                        av,
                                        v3[:, c - 1:c + 1, hA + hi, :],
                                        at2[:, :, hi, :],
                                        start=(cp2 == 0),
                                        stop=(cp2 == NL // 2 - 1),
                                        perf_mode=W8)
                        else:
                            for hi, av in enumerate((avA, avB)):
                                nc.tensor.matmul(
                                    av, v3[:, c, hA + hi, :],
                                    at2[:, ci, hi, :],
                                    start=(c == 0), stop=(c == NL - 1))
                        if t == 0 and c <= 1:
                            dump_tile(f"at0{c}", at2[:, ci, :, :])
                        if c == NORM_AT and t - 1 in av_tiles:
                            emit_norm(t - 1)
                        if c == 3 and mid_fn is not None:
                            mid_fn()

                emit_proj(0)
                for t in range(HP):
                    if t < HP - 1:
                        emit_attn(t, mid_fn=lambda t=t: emit_proj(t + 1))
                    else:
                        emit_attn(t)
                emit_norm(HP - 1)
                dump_tile("v3", v3[:, :, :, :])
                dump_tile("oT", oT[:, :, :])

                xres_sb = yop.tile([PT, 4, L], F32, tag="xres", bufs=1)
                for qb in range(4):
                    nc.sync.dma_start(xres_sb[:, qb, :], xres_d[:, qb, :])

                # ========== out-projection + residual ==========
                for qb in range(4):
                    qsl = slice(qb * PT, (qb + 1) * PT)
                    xr_sb = xres_sb[:, qb, :]
                    y_sb = yop.tile([PT, E], F32, tag="y")
                    for eh in range(2):
                        esl = slice(eh * QL, (eh + 1) * QL)
                        if VW == PT:
                            psF = avp.tile([VW, QL], F32, name="psF",
                                           tag="avA" if (2 * qb + eh) % 2 == 0
                                           else "avB")
                        else:
                            psF = psp.tile([PT, QL], F32, tag="ps0",
                                           name="psF")
                        if FP8_OUT:
                            for m in range(NE // 2):
                                nc.tensor.matmul(
                                    psF, oT[:, 2 * m:2 * m + 2, qsl],
                                    wo_sb[:, 2 * m:2 * m + 2, esl],
                                    start=(m == 0),
                                    stop=(m == NE // 2 - 1
                                          and "o" not in pbr),
                                    perf_mode=W8)
                        else:
                            for m in range(NE):
                                nc.tensor.matmul(
                                    psF, oT[:, m, qsl], wo_sb[:, m, esl],
                                    start=(m == 0),
                                    stop=(m == NE - 1 and "o" not in pbr))
                        if "o" in pbr:
                            nc.tensor.matmul(psF, ones_row[0:1, 0:1],
                                             pbr["o"][:, esl],
                                             start=False, stop=True)
                        if FP8_OUT:
                            nc.vector.scalar_tensor_tensor(
                                y_sb[:, esl], psF, 1.0 / (WSCALE * WSCALE),
                                xr_sb[:, esl],
                                op0=mybir.AluOpType.mult,
                                op1=mybir.AluOpType.add)
                        else:
                            nc.vector.tensor_add(y_sb[:, esl], psF,
                                                 xr_sb[:, esl])
                    nc.sync.dma_start(y_d[qsl, :], y_sb)
    return nc


def _prep_inputs(x, bias, mask, wq, bq, wk, bk, wv, bv, wo, bo, gate,
                 ln_g, ln_b):
    gate = np.asarray(gate, np.float32)
    ln_g = np.asarray(ln_g, np.float32)
    ln_b = np.asarray(ln_b, np.float32)
    grep = np.repeat(gate, D)
    safe_gate = bool(np.all(np.abs(gate) > 1e-6))
    if safe_gate:
        qscale = (SCALE / grep).astype(np.float32)
        exp_scales = [float(g) for g in gate]
    else:
        qscale = np.full(E, SCALE, np.float32)
        exp_scales = [1.0] * H

    sq = WSCALE if FP8_QKPROJ else 1.0
    sv = WSCALE if FP8_V else 1.0
    so = WSCALE if FP8_OUT else 1.0
    wqt = (np.asarray(wq).T * ln_g[:, None] * qscale[None, :] * sq)
    wkt = (np.asarray(wk).T * ln_g[:, None] * sq)
    wvt = (np.asarray(wv).T * ln_g[:, None] * sv)
    wot = (np.asarray(wo).T * so)
    bqe = ((np.asarray(wq) @ ln_b + np.asarray(bq)) * qscale * sq)
    bke = ((np.asarray(wk) @ ln_b + np.asarray(bk)) * sq)
    bve = ((np.asarray(wv) @ ln_b + np.asarray(bv)) * sv)
    # out-proj psum is divided by WSCALE^2 when FP8_OUT (oT and wo both
    # carry WSCALE); residual x is pre-multiplied to compensate.
    boe = np.asarray(bo, np.float32) * (so * so if FP8_OUT else 1.0)
    use_pbias = tuple(bool(np.any(b)) for b in (bqe, bke, bve, boe))

    mask = np.asarray(mask, np.int32)
    use_mask = not bool(np.all(mask == 1))

    def wfmt(w, f8):
        w = np.ascontiguousarray(w.reshape(NE, PT, E).transpose(1, 0, 2))
        return w.astype(FP8_NP if f8 else BF_NP)

    shared = {"wqt": wfmt(wqt, FP8_QKPROJ), "wkt": wfmt(wkt, FP8_QKPROJ),
              "wvt": wfmt(wvt, FP8_V), "wot": wfmt(wot, FP8_OUT)}
    for name, use, b in zip("qkvo", use_pbias, (bqe, bke, bve, boe)):
        if use:
            shared[f"b{name}e"] = b.reshape(1, E).astype(BF_NP)

    x = np.asarray(x, np.float32)
    bias = np.asarray(bias, np.float32)
    in_maps = []
    for c in range(NCORES):
        b_idx, qh = divmod(c, 2)
        q0 = qh * QL
        xr = np.roll(x[b_idx], -q0, axis=0)
        m = {}
        m.update(shared)
        m["xcb"] = np.ascontiguousarray(
            xr.reshape(NL, PT, L).transpose(1, 0, 2)).astype(BF_NP)
        m["xtc"] = np.ascontiguousarray(
            xr.T.reshape(NE, PT, L).transpose(1, 0, 2)).astype(BF_NP)
        m["xres"] = np.ascontiguousarray(
            xr[:QL].reshape(4, PT, L).transpose(1, 0, 2)).astype(np.float32)
        bs = bias[b_idx][:, q0:q0 + QL, :]      # [H, QL, L]
        bs = np.roll(bs, -q0, axis=2)           # roll key axis
        if not safe_gate:
            bs = bs * gate[:, None, None]
        bs = np.ascontiguousarray(bs.transpose(2, 0, 1))  # [L, H, QL]
        m["biasc"] = bs.astype(FP8_NP if FP8_BIAS else BF_NP)
        if use_mask:
            mr = np.roll(mask[b_idx], -q0)
            kmf = (-10000.0 * (1.0 - mr.astype(np.float32))) + SHIFT
            m["kmc"] = np.ascontiguousarray(
                kmf.reshape(NL, PT).T).astype(np.float32)
            m["mqc"] = mr[:QL].astype(np.float32).reshape(1, QL)
        in_maps.append(m)
    return in_maps, (exp_scales, use_pbias, use_mask)


def kernel(**inputs):
    global LAST_RESULT
    in_maps, (exp_scales, use_pbias, use_mask) = _prep_inputs(**inputs)
    nc = _build_nc(exp_scales, use_pbias, use_mask)
    if not nc.is_finalized():
        nc.finalize()
    res = run_bass_kernel_spmd(nc, in_maps, core_ids=list(range(NCORES)))
    LAST_RESULT = res
    out = np.empty((B, L, E), np.float32)
    for c in range(NCORES):
        b_idx, qh = divmod(c, 2)
        out[b_idx, qh * QL:(qh + 1) * QL, :] = res.results[c]["yc"]
    return out


# revision 77
# speedup vs baseline: 1.1599x; 1.1599x over previous
"""BiasedMultiHeadAttention Trainium2 kernel (restructured).

Sharding: 8 cores = (batch b, query-half qh); per-core x rows host-rolled so
the query block is rows 0..511 -> one SPMD program for all cores.

Structure vs baseline:
- x is host-transposed (xtc = x^T); LayerNorm applied as a column affine
  (xn^T = x^T*r_row - (mu*r)_row) using PE-transposed stats rows broadcast
  across partitions.  No on-device transposes of x, no serial LN->transpose
  startup.
- Projections and attention interleaved per head-pair t: emit attn(t-1)
  then proj(t).  K/Q tiles are per-pair pool tiles (precise deps).
- Softmax bias-add split across DVE (head A) and Pool (head B).
- Normalization (rowsum reciprocal) has no DRAM roundtrip
  (gpsimd.partition_broadcast) and is software-pipelined into the next
  pair's chunk loop.
- Optional fp8 DoubleRow paths (Q/K projections, V/AV, out-proj) and fp8
  bias DMA.
"""

import numpy as np
import ml_dtypes

import concourse.bass as bass
import concourse.tile as tile
import concourse.mybir as mybir
from concourse import bacc
from concourse.bass_utils import run_bass_kernel_spmd

B, L, E, H = 4, 1024, 1024, 16
D = E // H
SCALE = D**-0.5
EPS = 1e-5
NCORES = 8
QL = 512
PT = 128
NL = L // PT
NE = E // PT
HP = H // 2

F32 = mybir.dt.float32
BF16 = mybir.dt.bfloat16
FP8 = mybir.dt.float8e4
BF_NP = ml_dtypes.bfloat16
FP8_NP = ml_dtypes.float8_e4m3

# ---- feature flags ----
FP8_QKPROJ = True    # Q/K projections in fp8 DoubleRow
FP8_V = True         # V projection in fp8 DoubleRow
FP8_AV = True        # attn weights + V in fp8, DoubleRow AV matmuls
FP8_BIAS = True      # bias tensor DMA'd as fp8
FP8_OUT = True       # out-projection in fp8 DoubleRow
WSCALE = 32.0        # host premultiplier for fp8 weights (sigma -> ~1)
SHIFT = -1.0         # exp shift (fp8 attn-weight range centering)
MM_ADD = (0, 1, 2, 3, 4, 5, 6, 7)  # chunks whose bias add is a PE identity-matmul
NORM_AT = 1          # chunk index inside pair t at which norm(t-1) is emitted

LAST_RESULT = None


def _build_nc(gates, use_pbias, use_mask, dump=()):
    nc = bacc.Bacc("TRN2", target_bir_lowering=False, debug=False)
    same_gate = len(set(gates)) == 1

    def dump_tile(name, ap):
        if name in dump:
            d = nc.dram_tensor("d_" + name, list(ap.shape), ap.dtype,
                               kind="ExternalOutput")
            nc.sync.dma_start(d[tuple(slice(None) for _ in ap.shape)], ap)
    PRJ8 = FP8_QKPROJ
    W8 = mybir.MatmulPerfMode.DoubleRow

    xcb_d = nc.dram_tensor("xcb", [PT, NL, L], BF16, kind="ExternalInput")
    xt_d = nc.dram_tensor("xtc", [PT, NE, L], BF16, kind="ExternalInput")
    xres_d = nc.dram_tensor("xres", [PT, 4, L], F32, kind="ExternalInput")
    bias_d = nc.dram_tensor("biasc", [L, H, QL], FP8 if FP8_BIAS else BF16,
                            kind="ExternalInput")
    wq_d = nc.dram_tensor("wqt", [PT, NE, E], FP8 if PRJ8 else BF16,
                          kind="ExternalInput")
    wk_d = nc.dram_tensor("wkt", [PT, NE, E], FP8 if PRJ8 else BF16,
                          kind="ExternalInput")
    wv_d = nc.dram_tensor("wvt", [PT, NE, E], FP8 if FP8_V else BF16,
                          kind="ExternalInput")
    wo_d = nc.dram_tensor("wot", [PT, NE, E], FP8 if FP8_OUT else BF16,
                          kind="ExternalInput")
    pb_d = {}
    for name, use in zip("qkvo", use_pbias):
        if use:
            pb_d[name] = nc.dram_tensor(f"b{name}e", [1, E], BF16,
                                        kind="ExternalInput")
    if use_mask:
        km_d = nc.dram_tensor("kmc", [PT, NL], F32, kind="ExternalInput")
        mq_d = nc.dram_tensor("mqc", [1, QL], F32, kind="ExternalInput")
    y_d = nc.dram_tensor("yc", [QL, E], F32, kind="ExternalOutput")

    AT_DT = FP8 if FP8_AV else BF16
    OT_DT = FP8 if FP8_OUT else BF16
    V_DT = FP8 if FP8_AV else BF16
    # descale factors applied when copying projection psums to SBUF
    qk_dsc = (1.0 / WSCALE) if PRJ8 else None
    v_dsc = (1.0 / WSCALE) if FP8_V else None

    with tile.TileContext(nc) as tc:
        with (
            tc.tile_pool(name="persist", bufs=1) as pp,
            tc.tile_pool(name="consts", bufs=1) as cp,
        ):
            ident = cp.tile([PT, PT], BF16)
            nc.gpsimd.memset(ident, 0.0)
            nc.gpsimd.affine_select(
                out=ident, in_=ident,
                compare_op=mybir.AluOpType.not_equal, fill=1.0,
                base=0, pattern=[[-1, PT]], channel_multiplier=1)
            identb = ident
            if FP8_BIAS:
                identb = cp.tile([PT, PT], FP8)
                nc.gpsimd.tensor_copy(identb, ident)
            eps_t = cp.tile([PT, 1], F32)
            nc.vector.memset(eps_t, EPS)
            dummy = cp.tile([1, 2], F32)
            nc.vector.memset(dummy, 0.0)
            shift_t = None
            if FP8_AV:
                shift_t = cp.tile([PT, 1], F32)
                nc.vector.memset(shift_t, SHIFT)
            ones_row = None
            if pb_d:
                ones_row = cp.tile([1, QL], BF16)
                nc.vector.memset(ones_row, 1.0)
            if use_mask:
                km_sb = cp.tile([PT, NL], F32)
                nc.sync.dma_start(km_sb, km_d[:, :])
                mqrow = cp.tile([1, QL], F32)
                nc.sync.dma_start(mqrow, mq_d[:, :])
                mqb = cp.tile([64, QL], F32)
                nc.gpsimd.partition_broadcast(mqb, mqrow[0:1, :])

            # ---- persistent tensors ----
            # DMA priority: x chunks first (stats are the critical path),
            # then x^T, then weights in first-use order; wo last.
            xt_sb = pp.tile([PT, NE, L], BF16)
            wk_sb = pp.tile([PT, NE, E], FP8 if PRJ8 else BF16)
            wq_sb = pp.tile([PT, NE, E], FP8 if PRJ8 else BF16)
            wv_sb = pp.tile([PT, NE, E], FP8 if FP8_V else BF16)
            wo_sb = pp.tile([PT, NE, E], FP8 if FP8_OUT else BF16)
            pbr = {}
            for name in pb_d:
                pbr[name] = cp.tile([1, E], BF16)
                nc.sync.dma_start(pbr[name], pb_d[name][:, :])

            xnTb = pp.tile([PT, NE, L], BF16)   # normalized x^T
            xnT8 = None
            if PRJ8 or FP8_V:
                xnT8 = pp.tile([PT, NE, L], FP8)
            # V | ones col (| zero pad to 128 when DoubleRow AV: LDWEIGHTS
            # in DR mode requires 128-wide weight blocks)
            VW = 128 if FP8_AV else 65
            v3 = pp.tile([PT, NL, H, VW], V_DT)
            oT = pp.tile([PT, NE, QL], OT_DT)    # normalized attnout^T
            murb = pp.tile([PT, L], BF16)        # broadcast mu*r row
            rsrb = pp.tile([PT, L], BF16)        # broadcast r row

            # ========== Phase 0: stats ==========
            with (
                tc.tile_pool(name="st", bufs=2) as sp,
                tc.tile_pool(name="stp", bufs=1, space="PSUM") as stp,
            ):
                xcb_sb = sp.tile([PT, NL, L], BF16, tag="xcb", bufs=1)
                for lt in range(NL):
                    nc.sync.dma_start(xcb_sb[:, lt, :], xcb_d[:, lt, :])
                for ec in range(NE):
                    nc.sync.dma_start(xt_sb[:, ec, :], xt_d[:, ec, :])
                for ec in range(NE):
                    nc.sync.dma_start(wk_sb[:, ec, :], wk_d[:, ec, :])
                mvall = sp.tile([PT, NL, 2], F32, tag="mv")
                for lt in range(NL):
                    xr = xcb_sb[:, lt, :].rearrange("p (s d) -> p s d", s=2)
                    stats = sp.tile([PT, 2, 6], F32, tag="stats", bufs=3)
                    for sg in range(2):
                        nc.vector.bn_stats(stats[:, sg, :], xr[:, sg, :])
                    nc.vector.bn_aggr(mvall[:, lt, :], stats)
                sd = sp.tile([PT, NL], F32, tag="sd")
                nc.scalar.activation(sd, mvall[:, :, 1],
                                     mybir.ActivationFunctionType.Sqrt,
                                     bias=eps_t)
                # re-prime EXP after the sqrt so attention EXPs don't pay a
                # table reload
                nc.scalar.activation(dummy, sd[0:1, 0:2],
                                     mybir.ActivationFunctionType.Exp)
                rs = sp.tile([PT, NL], F32, tag="rs")
                nc.vector.reciprocal(rs, sd)
                pr2 = sp.tile([PT, 2, NL], BF16, tag="pr2")
                nc.vector.tensor_copy(pr2[:, 0, :], rs)
                nc.vector.tensor_mul(pr2[:, 1, :], mvall[:, :, 0], rs)
                psT = stp.tile([16, PT], BF16, tag="stT")
                nc.tensor.transpose(psT, pr2.rearrange("p a b -> p (a b)"),
                                    ident)
                prT = sp.tile([16, PT], BF16, tag="prT")
                nc.vector.tensor_copy(prT, psT)
                rsrow = sp.tile([1, L], BF16, tag="rsrow")
                nc.sync.dma_start(rsrow, prT[0:8, :])
                murow = sp.tile([1, L], BF16, tag="murow")
                nc.sync.dma_start(murow, prT[8:16, :])
                # remaining weights after the tiny row DMAs
                for ec in range(NE):
                    nc.sync.dma_start(wq_sb[:, ec, :], wq_d[:, ec, :])
                for ec in range(NE):
                    nc.sync.dma_start(wv_sb[:, ec, :], wv_d[:, ec, :])
                for ec in range(NE):
                    nc.sync.dma_start(wo_sb[:, ec, :], wo_d[:, ec, :])
                nc.gpsimd.partition_broadcast(rsrb, rsrow[0:1, :])
                nc.gpsimd.partition_broadcast(murb, murow[0:1, :])
                # v3 ones/zero-pad memsets after the broadcasts on Pool
                nc.gpsimd.memset(v3[:, :, :, 64:65], 1.0)
                if FP8_AV:
                    for lt in range(NL):
                        nc.gpsimd.memset(v3[:, lt, :, 65:128], 0.0)
                for lh in range(2):
                    lsl = slice(lh * QL, (lh + 1) * QL)
                    for ec in range(NE):
                        nc.vector.tensor_mul(xnTb[:, ec, lsl],
                                             xt_sb[:, ec, lsl], rsrb[:, lsl])
                        if PRJ8 and FP8_V:
                            nc.vector.tensor_sub(xnT8[:, ec, lsl],
                                                 xnTb[:, ec, lsl],
                                                 murb[:, lsl])
                        else:
                            nc.vector.tensor_sub(xnTb[:, ec, lsl],
                                                 xnTb[:, ec, lsl],
                                                 murb[:, lsl])
                            if xnT8 is not None:
                                nc.vector.tensor_copy(xnT8[:, ec, lsl],
                                                      xnTb[:, ec, lsl])
                dump_tile("rsrb", rsrb[:, :])
                dump_tile("murb", murb[:, :])
                dump_tile("xnTb", xnTb[:, :, :])

            # ========== interleaved projections + attention ==========
            with (
                tc.tile_pool(name="kq", bufs=1) as kqp,
                tc.tile_pool(name="ps", bufs=2, space="PSUM") as psp,
                tc.tile_pool(name="av", bufs=2, space="PSUM") as avp,
                tc.tile_pool(name="bias", bufs=8) as bp,
                tc.tile_pool(name="s1p", bufs=3) as s1p,
                tc.tile_pool(name="attn", bufs=3) as ap,
                tc.tile_pool(name="rbs", bufs=2) as rp,
                tc.tile_pool(name="oo", bufs=2) as oop,
                tc.tile_pool(name="yo", bufs=2) as yop,
            ):
                KBUFS = 3
                kq_tiles = {}   # t -> (kA, kB, qT)
                av_tiles = {}   # t -> (avA, avB)

                def qk_mm(ps, w, osl, nsl, extra=None, x8=False):
                    """accumulate ps += w[:, :, osl]^T @ xnT[:, :, nsl]"""
                    xn = xnT8 if x8 else xnTb
                    if x8:
                        for j in range(NE // 2):
                            nc.tensor.matmul(
                                ps, w[:, 2 * j:2 * j + 2, osl],
                                xn[:, 2 * j:2 * j + 2, nsl],
                                start=(j == 0),
                                stop=(j == NE // 2 - 1 and extra is None),
                                perf_mode=W8)
                    else:
                        for j in range(NE):
                            nc.tensor.matmul(
                                ps, w[:, j, osl], xn[:, j, nsl],
                                start=(j == 0),
                                stop=(j == NE - 1 and extra is None))
                    if extra is not None:
                        nc.tensor.matmul(ps, extra[:, osl],
                                         ones_row[:, 0:nsl.stop - nsl.start],
                                         start=False, stop=True)

                def emit_proj(t):
                    osl = slice(t * PT, (t + 1) * PT)
                    kA = kqp.tile([PT, L], BF16, tag="kA", bufs=KBUFS)
                    kB = kqp.tile([PT, L], BF16, tag="kB", bufs=KBUFS)
                    qT = kqp.tile([PT, QL], BF16, tag="qT", bufs=KBUFS)
                    if t < KBUFS:
                        nc.gpsimd.memset(kA[64:PT, :], 0.0)
                        nc.gpsimd.memset(kB[0:64, :], 0.0)
                    for nh in range(2):
                        nsl = slice(nh * QL, (nh + 1) * QL)
                        psK = psp.tile([PT, QL], F32, tag=f"ps{nh}")
                        qk_mm(psK, wk_sb, osl, nsl,
                              extra=pbr.get("k"), x8=PRJ8)
                        if qk_dsc is None:
                            nc.vector.tensor_copy(kA[0:64, nsl],
                                                  psK[0:64, :])
                            nc.vector.tensor_copy(kB[64:PT, nsl],
                                                  psK[64:PT, :])
                        else:
                            nc.vector.tensor_scalar_mul(
                                kA[0:64, nsl], psK[0:64, :], qk_dsc)
                            nc.vector.tensor_scalar_mul(
                                kB[64:PT, nsl], psK[64:PT, :], qk_dsc)
                    psQ = psp.tile([PT, QL], F32, tag="ps0")
                    qk_mm(psQ, wq_sb, osl, slice(0, QL),
                          extra=pbr.get("q"), x8=PRJ8)
                    if qk_dsc is None:
                        nc.scalar.copy(qT, psQ)
                    else:
                        nc.scalar.activation(
                            qT, psQ,
                            mybir.ActivationFunctionType.Copy,
                            scale=qk_dsc)
                    kq_tiles[t] = (kA, kB, qT)
                    if t == 0:
                        dump_tile("kA0", kA[:, :])
                        dump_tile("kB0", kB[:, :])
                        dump_tile("qT0", qT[:, :])
                    # V projection: emit half of V in t=0, other half in t=1
                    if t < 2:
                        vsl = slice(t * QL, (t + 1) * QL)
                        for lt in range(NL):
                            lsl = slice(lt * PT, (lt + 1) * PT)
                            psV = psp.tile([PT, QL], F32, tag="ps1",
                                           name="psV")
                            xn = xnT8 if FP8_V else xnTb
                            if FP8_V:
                                for j in range(NE // 2):
                                    nc.tensor.matmul(
                                        psV, xn[:, 2 * j:2 * j + 2, lsl],
                                        wv_sb[:, 2 * j:2 * j + 2, vsl],
                                        start=(j == 0),
                                        stop=(j == NE // 2 - 1
                                              and "v" not in pbr),
                                        perf_mode=W8)
                            else:
                                for j in range(NE):
                                    nc.tensor.matmul(
                                        psV, xn[:, j, lsl],
                                        wv_sb[:, j, vsl],
                                        start=(j == 0),
                                        stop=(j == NE - 1
                                              and "v" not in pbr))
                            if "v" in pbr:
                                nc.tensor.matmul(psV, ones_row[:, 0:PT],
                                                 pbr["v"][:, vsl],
                                                 start=False, stop=True)
                            dst = v3[:, lt, t * 8:(t + 1) * 8, 0:64]
                            src = psV.rearrange("p (h d) -> p h d", h=8)
                            eng = nc.vector if lt % 2 == 0 else nc.scalar
                            if v_dsc is None:
                                eng.tensor_copy(dst, src) \
                                    if eng is nc.vector else eng.copy(dst, src)
                            else:
                                if eng is nc.vector:
                                    eng.tensor_scalar_mul(dst, src, v_dsc)
                                else:
                                    eng.activation(
                                        dst, src,
                                        mybir.ActivationFunctionType.Copy,
                                        scale=v_dsc)

                def emit_norm(p):
                    avA, avB = av_tiles.pop(p)
                    osc = float(WSCALE) if FP8_OUT else 1.0
                    if p == 0 and "avA0" in dump:
                        avd = rp.tile([VW, QL], F32, tag="avd", bufs=1)
                        nc.vector.tensor_copy(avd, avA)
                        dump_tile("avA0", avd[:, :])
                    for hi, av in enumerate((avA, avB)):
                        rr = rp.tile([65, QL], F32, tag=f"rr{hi}", bufs=1)
                        nc.vector.tensor_copy(rr[64:65, :], av[64:65, :])
                        rr0 = rp.tile([1, QL], F32, tag=f"rr0{hi}", bufs=1)
                        nc.sync.dma_start(rr0, rr[64:65, :])
                        rsb = rp.tile([64, QL], F32, tag=f"rsb{hi}", bufs=1)
                        nc.gpsimd.partition_broadcast(rsb, rr0[0:1, :])
                        nc.vector.reciprocal_approx_fast(out=rsb, in_=rsb)
                        if use_mask:
                            nc.vector.tensor_mul(rsb, rsb, mqb)
                        if hi == 0:
                            nc.vector.scalar_tensor_tensor(
                                oT[0:64, p, :], av[0:64, :], osc, rsb,
                                op0=mybir.AluOpType.mult,
                                op1=mybir.AluOpType.mult)
                        else:
                            oto = oop.tile([64, QL], OT_DT, tag="oo")
                            nc.vector.scalar_tensor_tensor(
                                oto, av[0:64, :], osc, rsb,
                                op0=mybir.AluOpType.mult,
                                op1=mybir.AluOpType.mult)
                            nc.sync.dma_start(oT[64:PT, p, :], oto)

                def emit_attn(t):
                    kA, kB, qT = kq_tiles.pop(t)
                    hA, hB = 2 * t, 2 * t + 1
                    avA = avp.tile([VW, QL], F32, tag="avA")
                    avB = avp.tile([VW, QL], F32, tag="avB")
                    av_tiles[t] = (avA, avB)
                    at2 = None
                    for c in range(NL):
                        csl = slice(c * PT, (c + 1) * PT)
                        mm_add = c in MM_ADD
                        bt = bp.tile([PT, 2, QL], FP8 if FP8_BIAS else BF16,
                                     tag="bt")
                        nc.sync.dma_start(bt, bias_d[csl, hA:hB + 1, :])
                        if use_mask:
                            kmb = km_sb[:, c:c + 1]
                        elif FP8_AV:
                            kmb = shift_t
                        else:
                            kmb = 0.0
                        ci = c % 2
                        if ci == 0:
                            at2 = ap.tile([PT, 2, 2, QL], AT_DT, tag="at",
                                          bufs=4)
                        # separate per-head psum tiles + per-head EXPs keep
                        # the A-half pipeline independent of the B-half
                        for hi, kT in enumerate((kA, kB)):
                            psh = psp.tile([PT, QL], F32, tag=f"ps{hi}",
                                           name="psh")
                            nc.tensor.matmul(psh, kT[:, csl], qT,
                                             start=True, stop=not mm_add)
                            if mm_add:
                                nc.tensor.matmul(psh, identb, bt[:, hi, :],
                                                 start=False, stop=True)
                                s_in = psh
                            else:
                                s1 = s1p.tile([PT, QL], BF16, tag=f"s1{hi}")
                                nc.vector.tensor_add(s1, psh, bt[:, hi, :])
                                s_in = s1
                            nc.scalar.activation(
                                at2[:, ci, hi, :], s_in,
                                mybir.ActivationFunctionType.Exp,
                                bias=kmb, scale=gates[hA + hi])
                        if FP8_AV:
                            if ci == 1:
                                cp2 = c // 2
                                for hi, av in enumerate((avA, avB)):
                                    nc.tensor.matmul(
                                        av,
                                        v3[:, c - 1:c + 1, hA + hi, :],
                                        at2[:, :, hi, :],
                                        start=(cp2 == 0),
                                        stop=(cp2 == NL // 2 - 1),
                                        perf_mode=W8)
                        else:
                            for hi, av in enumerate((avA, avB)):
                                nc.tensor.matmul(
                                    av, v3[:, c, hA + hi, :],
                                    at2[:, ci, hi, :],
                                    start=(c == 0), stop=(c == NL - 1))
                        if t == 0 and c <= 1:
                            dump_tile(f"at0{c}", at2[:, ci, :, :])
                        if c == NORM_AT and t - 1 in av_tiles:
                            emit_norm(t - 1)

                emit_proj(0)
                for t in range(1, HP):
                    emit_attn(t - 1)
                    emit_proj(t)
                emit_attn(HP - 1)
                emit_norm(HP - 1)
                dump_tile("v3", v3[:, :, :, :])
                dump_tile("oT", oT[:, :, :])

                xres_sb = yop.tile([PT, 4, L], F32, tag="xres", bufs=1)
                for qb in range(4):
                    nc.sync.dma_start(xres_sb[:, qb, :], xres_d[:, qb, :])

                # ========== out-projection + residual ==========
                for qb in range(4):
                    qsl = slice(qb * PT, (qb + 1) * PT)
                    xr_sb = xres_sb[:, qb, :]
                    y_sb = yop.tile([PT, E], F32, tag="y")
                    for eh in range(2):
                        esl = slice(eh * QL, (eh + 1) * QL)
                        if VW == PT:
                            psF = avp.tile([VW, QL], F32, name="psF",
                                           tag="avA" if (2 * qb + eh) % 2 == 0
                                           else "avB")
                        else:
                            psF = psp.tile([PT, QL], F32, tag="ps0",
                                           name="psF")
                        if FP8_OUT:
                            for m in range(NE // 2):
                                nc.tensor.matmul(
                                    psF, oT[:, 2 * m:2 * m + 2, qsl],
                                    wo_sb[:, 2 * m:2 * m + 2, esl],
                                    start=(m == 0),
                                    stop=(m == NE // 2 - 1
                                          and "o" not in pbr),
                                    perf_mode=W8)
                        else:
                            for m in range(NE):
                                nc.tensor.matmul(
                                    psF, oT[:, m, qsl], wo_sb[:, m, esl],
                                    start=(m == 0),
                                    stop=(m == NE - 1 and "o" not in pbr))
                        if "o" in pbr:
                            nc.tensor.matmul(psF, ones_row[0:1, 0:1],
                                             pbr["o"][:, esl],
                                             start=False, stop=True)
                        if FP8_OUT:
                            nc.vector.scalar_tensor_tensor(
                                y_sb[:, esl], psF, 1.0 / (WSCALE * WSCALE),
                                xr_sb[:, esl],
                                op0=mybir.AluOpType.mult,
                                op1=mybir.AluOpType.add)
                        else:
                            nc.vector.tensor_add(y_sb[:, esl], psF,
                                                 xr_sb[:, esl])
                    nc.sync.dma_start(y_d[qsl, :], y_sb)
    return nc


def _prep_inputs(x, bias, mask, wq, bq, wk, bk, wv, bv, wo, bo, gate,
                 ln_g, ln_b):
    gate = np.asarray(gate, np.float32)
    ln_g = np.asarray(ln_g, np.float32)
    ln_b = np.asarray(ln_b, np.float32)
    grep = np.repeat(gate, D)
    safe_gate = bool(np.all(np.abs(gate) > 1e-6))
    if safe_gate:
        qscale = (SCALE / grep).astype(np.float32)
        exp_scales = [float(g) for g in gate]
    else:
        qscale = np.full(E, SCALE, np.float32)
        exp_scales = [1.0] * H

    sq = WSCALE if FP8_QKPROJ else 1.0
    sv = WSCALE if FP8_V else 1.0
    so = WSCALE if FP8_OUT else 1.0
    wqt = (np.asarray(wq).T * ln_g[:, None] * qscale[None, :] * sq)
    wkt = (np.asarray(wk).T * ln_g[:, None] * sq)
    wvt = (np.asarray(wv).T * ln_g[:, None] * sv)
    wot = (np.asarray(wo).T * so)
    bqe = ((np.asarray(wq) @ ln_b + np.asarray(bq)) * qscale * sq)
    bke = ((np.asarray(wk) @ ln_b + np.asarray(bk)) * sq)
    bve = ((np.asarray(wv) @ ln_b + np.asarray(bv)) * sv)
    # out-proj psum is divided by WSCALE^2 when FP8_OUT (oT and wo both
    # carry WSCALE); residual x is pre-multiplied to compensate.
    boe = np.asarray(bo, np.float32) * (so * so if FP8_OUT else 1.0)
    use_pbias = tuple(bool(np.any(b)) for b in (bqe, bke, bve, boe))

    mask = np.asarray(mask, np.int32)
    use_mask = not bool(np.all(mask == 1))

    def wfmt(w, f8):
        w = np.ascontiguousarray(w.reshape(NE, PT, E).transpose(1, 0, 2))
        return w.astype(FP8_NP if f8 else BF_NP)

    shared = {"wqt": wfmt(wqt, FP8_QKPROJ), "wkt": wfmt(wkt, FP8_QKPROJ),
              "wvt": wfmt(wvt, FP8_V), "wot": wfmt(wot, FP8_OUT)}
    for name, use, b in zip("qkvo", use_pbias, (bqe, bke, bve, boe)):
        if use:
            shared[f"b{name}e"] = b.reshape(1, E).astype(BF_NP)

    x = np.asarray(x, np.float32)
    bias = np.asarray(bias, np.float32)
    in_maps = []
    for c in range(NCORES):
        b_idx, qh = divmod(c, 2)
        q0 = qh * QL
        xr = np.roll(x[b_idx], -q0, axis=0)
        m = {}
        m.update(shared)
        m["xcb"] = np.ascontiguousarray(
            xr.reshape(NL, PT, L).transpose(1, 0, 2)).astype(BF_NP)
        m["xtc"] = np.ascontiguousarray(
            xr.T.reshape(NE, PT, L).transpose(1, 0, 2)).astype(BF_NP)
        m["xres"] = np.ascontiguousarray(
            xr[:QL].reshape(4, PT, L).transpose(1, 0, 2)).astype(np.float32)
        bs = bias[b_idx][:, q0:q0 + QL, :]      # [H, QL, L]
        bs = np.roll(bs, -q0, axis=2)           # roll key axis
        if not safe_gate:
            bs = bs * gate[:, None, None]
        bs = np.ascontiguousarray(bs.transpose(2, 0, 1))  # [L, H, QL]
        m["biasc"] = bs.astype(FP8_NP if FP8_BIAS else BF_NP)
        if use_mask:
            mr = np.roll(mask[b_idx], -q0)
            kmf = (-10000.0 * (1.0 - mr.astype(np.float32))) + SHIFT
            m["kmc"] = np.ascontiguousarray(
                kmf.reshape(NL, PT).T).astype(np.float32)
            m["mqc"] = mr[:QL].astype(np.float32).reshape(1, QL)
        in_maps.append(m)
    return in_maps, (exp_scales, use_pbias, use_mask)


def kernel(**inputs):
    global LAST_RESULT
    in_maps, (exp_scales, use_pbias, use_mask) = _prep_inputs(**inputs)
    nc = _build_nc(exp_scales, use_pbias, use_mask)
    if not nc.is_finalized():
        nc.finalize()
    res = run_bass_kernel_spmd(nc, in_maps, core_ids=list(range(NCORES)))
    LAST_RESULT = res
    out = np.empty((B, L, E), np.float32)
    for c in range(NCORES):
        b_idx, qh = divmod(c, 2)
        out[b_idx, qh * QL:(qh + 1) * QL, :] = res.results[c]["yc"]
    return out


# revision 78
# speedup vs baseline: 1.1806x; 1.0179x over previous
"""BiasedMultiHeadAttention Trainium2 kernel (restructured).

Sharding: 8 cores = (batch b, query-half qh); per-core x rows host-rolled so
the query block is rows 0..511 -> one SPMD program for all cores.

Structure vs baseline:
- x is host-transposed (xtc = x^T); LayerNorm applied as a column affine
  (xn^T = x^T*r_row - (mu*r)_row) using PE-transposed stats rows broadcast
  across partitions.  No on-device transposes of x, no serial LN->transpose
  startup.
- Projections and attention interleaved per head-pair t: emit attn(t-1)
  then proj(t).  K/Q tiles are per-pair pool tiles (precise deps).
- Softmax bias-add split across DVE (head A) and Pool (head B).
- Normalization (rowsum reciprocal) has no DRAM roundtrip
  (gpsimd.partition_broadcast) and is software-pipelined into the next
  pair's chunk loop.
- Optional fp8 DoubleRow paths (Q/K projections, V/AV, out-proj) and fp8
  bias DMA.
"""

import numpy as np
import ml_dtypes

import concourse.bass as bass
import concourse.tile as tile
import concourse.mybir as mybir
from concourse import bacc
from concourse.bass_utils import run_bass_kernel_spmd

B, L, E, H = 4, 1024, 1024, 16
D = E // H
SCALE = D**-0.5
EPS = 1e-5
NCORES = 8
QL = 512
PT = 128
NL = L // PT
NE = E // PT
HP = H // 2

F32 = mybir.dt.float32
BF16 = mybir.dt.bfloat16
FP8 = mybir.dt.float8e4
BF_NP = ml_dtypes.bfloat16
FP8_NP = ml_dtypes.float8_e4m3

# ---- feature flags ----
FP8_QKPROJ = True    # Q/K projections in fp8 DoubleRow
FP8_V = True         # V projection in fp8 DoubleRow
FP8_AV = True        # attn weights + V in fp8, DoubleRow AV matmuls
FP8_BIAS = True      # bias tensor DMA'd as fp8
FP8_OUT = True       # out-projection in fp8 DoubleRow
WSCALE = 32.0        # host premultiplier for fp8 weights (sigma -> ~1)
SHIFT = -1.0         # exp shift (fp8 attn-weight range centering)
MM_ADD = (0, 1, 2, 3, 4, 5, 6, 7)  # chunks whose bias add is a PE identity-matmul
NORM_AT = 1          # chunk index inside pair t at which norm(t-1) is emitted

LAST_RESULT = None


def _build_nc(gates, use_pbias, use_mask, dump=()):
    nc = bacc.Bacc("TRN2", target_bir_lowering=False, debug=False)
    same_gate = len(set(gates)) == 1

    def dump_tile(name, ap):
        if name in dump:
            d = nc.dram_tensor("d_" + name, list(ap.shape), ap.dtype,
                               kind="ExternalOutput")
            nc.sync.dma_start(d[tuple(slice(None) for _ in ap.shape)], ap)
    PRJ8 = FP8_QKPROJ
    W8 = mybir.MatmulPerfMode.DoubleRow

    xcb_d = nc.dram_tensor("xcb", [PT, NL, L], BF16, kind="ExternalInput")
    xt_d = nc.dram_tensor("xtc", [PT, NE, L], BF16, kind="ExternalInput")
    xres_d = nc.dram_tensor("xres", [PT, 4, L], F32, kind="ExternalInput")
    bias_d = nc.dram_tensor("biasc", [L, H, QL], FP8 if FP8_BIAS else BF16,
                            kind="ExternalInput")
    wq_d = nc.dram_tensor("wqt", [PT, NE, E], FP8 if PRJ8 else BF16,
                          kind="ExternalInput")
    wk_d = nc.dram_tensor("wkt", [PT, NE, E], FP8 if PRJ8 else BF16,
                          kind="ExternalInput")
    wv_d = nc.dram_tensor("wvt", [PT, NE, E], FP8 if FP8_V else BF16,
                          kind="ExternalInput")
    wo_d = nc.dram_tensor("wot", [PT, NE, E], FP8 if FP8_OUT else BF16,
                          kind="ExternalInput")
    pb_d = {}
    for name, use in zip("qkvo", use_pbias):
        if use:
            pb_d[name] = nc.dram_tensor(f"b{name}e", [1, E], BF16,
                                        kind="ExternalInput")
    if use_mask:
        km_d = nc.dram_tensor("kmc", [PT, NL], F32, kind="ExternalInput")
        mq_d = nc.dram_tensor("mqc", [1, QL], F32, kind="ExternalInput")
    y_d = nc.dram_tensor("yc", [QL, E], F32, kind="ExternalOutput")

    AT_DT = FP8 if FP8_AV else BF16
    OT_DT = FP8 if FP8_OUT else BF16
    V_DT = FP8 if FP8_AV else BF16
    # descale factors applied when copying projection psums to SBUF
    qk_dsc = (1.0 / WSCALE) if PRJ8 else None
    v_dsc = (1.0 / WSCALE) if FP8_V else None

    with tile.TileContext(nc) as tc:
        with (
            tc.tile_pool(name="persist", bufs=1) as pp,
            tc.tile_pool(name="consts", bufs=1) as cp,
        ):
            ident = cp.tile([PT, PT], BF16)
            nc.gpsimd.memset(ident, 0.0)
            nc.gpsimd.affine_select(
                out=ident, in_=ident,
                compare_op=mybir.AluOpType.not_equal, fill=1.0,
                base=0, pattern=[[-1, PT]], channel_multiplier=1)
            identb = ident
            if FP8_BIAS:
                identb = cp.tile([PT, PT], FP8)
                nc.gpsimd.tensor_copy(identb, ident)
            eps_t = cp.tile([PT, 1], F32)
            nc.vector.memset(eps_t, EPS)
            dummy = cp.tile([1, 2], F32)
            nc.vector.memset(dummy, 0.0)
            shift_t = None
            if FP8_AV:
                shift_t = cp.tile([PT, 1], F32)
                nc.vector.memset(shift_t, SHIFT)
            ones_row = None
            if pb_d:
                ones_row = cp.tile([1, QL], BF16)
                nc.vector.memset(ones_row, 1.0)
            if use_mask:
                km_sb = cp.tile([PT, NL], F32)
                nc.sync.dma_start(km_sb, km_d[:, :])
                mqrow = cp.tile([1, QL], F32)
                nc.sync.dma_start(mqrow, mq_d[:, :])
                mqb = cp.tile([64, QL], F32)
                nc.gpsimd.partition_broadcast(mqb, mqrow[0:1, :])

            # ---- persistent tensors ----
            # DMA priority: x chunks first (stats are the critical path),
            # then x^T, then weights in first-use order; wo last.
            xt_sb = pp.tile([PT, NE, L], BF16)
            wk_sb = pp.tile([PT, NE, E], FP8 if PRJ8 else BF16)
            wq_sb = pp.tile([PT, NE, E], FP8 if PRJ8 else BF16)
            wv_sb = pp.tile([PT, NE, E], FP8 if FP8_V else BF16)
            wo_sb = pp.tile([PT, NE, E], FP8 if FP8_OUT else BF16)
            pbr = {}
            for name in pb_d:
                pbr[name] = cp.tile([1, E], BF16)
                nc.sync.dma_start(pbr[name], pb_d[name][:, :])

            xnTb = pp.tile([PT, NE, L], BF16)   # normalized x^T
            xnT8 = None
            if PRJ8 or FP8_V:
                xnT8 = pp.tile([PT, NE, L], FP8)
            # V | ones col (| zero pad to 128 when DoubleRow AV: LDWEIGHTS
            # in DR mode requires 128-wide weight blocks)
            VW = 128 if FP8_AV else 65
            v3 = pp.tile([PT, NL, H, VW], V_DT)
            oT = pp.tile([PT, NE, QL], OT_DT)    # normalized attnout^T
            murb = pp.tile([PT, L], BF16)        # broadcast mu*r row
            rsrb = pp.tile([PT, L], BF16)        # broadcast r row

            # ========== Phase 0: stats ==========
            with (
                tc.tile_pool(name="st", bufs=2) as sp,
                tc.tile_pool(name="stp", bufs=1, space="PSUM") as stp,
            ):
                xcb_sb = sp.tile([PT, NL, L], BF16, tag="xcb", bufs=1)
                for lt in range(NL):
                    nc.sync.dma_start(xcb_sb[:, lt, :], xcb_d[:, lt, :])
                for ec in range(NE):
                    nc.sync.dma_start(xt_sb[:, ec, :], xt_d[:, ec, :])
                for ec in range(NE):
                    nc.sync.dma_start(wk_sb[:, ec, :], wk_d[:, ec, :])
                mvall = sp.tile([PT, NL, 2], F32, tag="mv")
                for lt in range(NL):
                    xr = xcb_sb[:, lt, :].rearrange("p (s d) -> p s d", s=2)
                    stats = sp.tile([PT, 2, 6], F32, tag="stats", bufs=3)
                    for sg in range(2):
                        nc.vector.bn_stats(stats[:, sg, :], xr[:, sg, :])
                    nc.vector.bn_aggr(mvall[:, lt, :], stats)
                sd = sp.tile([PT, NL], F32, tag="sd")
                nc.scalar.activation(sd, mvall[:, :, 1],
                                     mybir.ActivationFunctionType.Sqrt,
                                     bias=eps_t)
                # re-prime EXP after the sqrt so attention EXPs don't pay a
                # table reload
                nc.scalar.activation(dummy, sd[0:1, 0:2],
                                     mybir.ActivationFunctionType.Exp)
                rs = sp.tile([PT, NL], F32, tag="rs")
                nc.vector.reciprocal(rs, sd)
                pr2 = sp.tile([PT, 2, NL], BF16, tag="pr2")
                nc.vector.tensor_copy(pr2[:, 0, :], rs)
                nc.vector.tensor_mul(pr2[:, 1, :], mvall[:, :, 0], rs)
                psT = stp.tile([16, PT], BF16, tag="stT")
                nc.tensor.transpose(psT, pr2.rearrange("p a b -> p (a b)"),
                                    ident)
                prT = sp.tile([16, PT], BF16, tag="prT")
                nc.vector.tensor_copy(prT, psT)
                rsrow = sp.tile([1, L], BF16, tag="rsrow")
                nc.sync.dma_start(rsrow, prT[0:8, :])
                murow = sp.tile([1, L], BF16, tag="murow")
                nc.sync.dma_start(murow, prT[8:16, :])
                # remaining weights after the tiny row DMAs
                for ec in range(NE):
                    nc.sync.dma_start(wq_sb[:, ec, :], wq_d[:, ec, :])
                for ec in range(NE):
                    nc.sync.dma_start(wv_sb[:, ec, :], wv_d[:, ec, :])
                for ec in range(NE):
                    nc.sync.dma_start(wo_sb[:, ec, :], wo_d[:, ec, :])
                nc.gpsimd.partition_broadcast(rsrb, rsrow[0:1, :])
                nc.gpsimd.partition_broadcast(murb, murow[0:1, :])
                # v3 ones/zero-pad memsets after the broadcasts on Pool
                nc.gpsimd.memset(v3[:, :, :, 64:65], 1.0)
                if FP8_AV:
                    for lt in range(NL):
                        nc.gpsimd.memset(v3[:, lt, :, 65:128], 0.0)
                for lh in range(2):
                    lsl = slice(lh * QL, (lh + 1) * QL)
                    for ec in range(NE):
                        nc.vector.tensor_mul(xnTb[:, ec, lsl],
                                             xt_sb[:, ec, lsl], rsrb[:, lsl])
                        if PRJ8 and FP8_V:
                            # bf16 subtract runs in DVE 2x mode; the fp8
                            # cast rides the idle ACT engine
                            nc.vector.tensor_sub(xnTb[:, ec, lsl],
                                                 xnTb[:, ec, lsl],
                                                 murb[:, lsl])
                            nc.scalar.copy(xnT8[:, ec, lsl],
                                           xnTb[:, ec, lsl])
                        else:
                            nc.vector.tensor_sub(xnTb[:, ec, lsl],
                                                 xnTb[:, ec, lsl],
                                                 murb[:, lsl])
                            if xnT8 is not None:
                                nc.vector.tensor_copy(xnT8[:, ec, lsl],
                                                      xnTb[:, ec, lsl])
                dump_tile("rsrb", rsrb[:, :])
                dump_tile("murb", murb[:, :])
                dump_tile("xnTb", xnTb[:, :, :])

            # ========== interleaved projections + attention ==========
            with (
                tc.tile_pool(name="kq", bufs=1) as kqp,
                tc.tile_pool(name="ps", bufs=2, space="PSUM") as psp,
                tc.tile_pool(name="av", bufs=2, space="PSUM") as avp,
                tc.tile_pool(name="bias", bufs=8) as bp,
                tc.tile_pool(name="s1p", bufs=3) as s1p,
                tc.tile_pool(name="attn", bufs=3) as ap,
                tc.tile_pool(name="rbs", bufs=2) as rp,
                tc.tile_pool(name="oo", bufs=2) as oop,
                tc.tile_pool(name="yo", bufs=2) as yop,
            ):
                KBUFS = 3
                kq_tiles = {}   # t -> (kA, kB, qT)
                av_tiles = {}   # t -> (avA, avB)

                def qk_mm(ps, w, osl, nsl, extra=None, x8=False):
                    """accumulate ps += w[:, :, osl]^T @ xnT[:, :, nsl]"""
                    xn = xnT8 if x8 else xnTb
                    if x8:
                        for j in range(NE // 2):
                            nc.tensor.matmul(
                                ps, w[:, 2 * j:2 * j + 2, osl],
                                xn[:, 2 * j:2 * j + 2, nsl],
                                start=(j == 0),
                                stop=(j == NE // 2 - 1 and extra is None),
                                perf_mode=W8)
                    else:
                        for j in range(NE):
                            nc.tensor.matmul(
                                ps, w[:, j, osl], xn[:, j, nsl],
                                start=(j == 0),
                                stop=(j == NE - 1 and extra is None))
                    if extra is not None:
                        nc.tensor.matmul(ps, extra[:, osl],
                                         ones_row[:, 0:nsl.stop - nsl.start],
                                         start=False, stop=True)

                def emit_proj(t):
                    osl = slice(t * PT, (t + 1) * PT)
                    kA = kqp.tile([PT, L], BF16, tag="kA", bufs=KBUFS)
                    kB = kqp.tile([PT, L], BF16, tag="kB", bufs=KBUFS)
                    qT = kqp.tile([PT, QL], BF16, tag="qT", bufs=KBUFS)
                    if t < KBUFS:
                        nc.gpsimd.memset(kA[64:PT, :], 0.0)
                        nc.gpsimd.memset(kB[0:64, :], 0.0)
                    for nh in range(2):
                        nsl = slice(nh * QL, (nh + 1) * QL)
                        psK = psp.tile([PT, QL], F32, tag=f"ps{nh}")
                        qk_mm(psK, wk_sb, osl, nsl,
                              extra=pbr.get("k"), x8=PRJ8)
                        if qk_dsc is None:
                            nc.vector.tensor_copy(kA[0:64, nsl],
                                                  psK[0:64, :])
                            nc.vector.tensor_copy(kB[64:PT, nsl],
                                                  psK[64:PT, :])
                        else:
                            nc.vector.tensor_scalar_mul(
                                kA[0:64, nsl], psK[0:64, :], qk_dsc)
                            nc.vector.tensor_scalar_mul(
                                kB[64:PT, nsl], psK[64:PT, :], qk_dsc)
                    psQ = psp.tile([PT, QL], F32, tag="ps0")
                    qk_mm(psQ, wq_sb, osl, slice(0, QL),
                          extra=pbr.get("q"), x8=PRJ8)
                    if qk_dsc is None:
                        nc.scalar.copy(qT, psQ)
                    else:
                        nc.scalar.activation(
                            qT, psQ,
                            mybir.ActivationFunctionType.Copy,
                            scale=qk_dsc)
                    kq_tiles[t] = (kA, kB, qT)
                    if t == 0:
                        dump_tile("kA0", kA[:, :])
                        dump_tile("kB0", kB[:, :])
                        dump_tile("qT0", qT[:, :])
                    # V projection: emit half of V in t=0, other half in t=1
                    if t < 2:
                        vsl = slice(t * QL, (t + 1) * QL)
                        for lt in range(NL):
                            lsl = slice(lt * PT, (lt + 1) * PT)
                            psV = psp.tile([PT, QL], F32, tag="ps1",
                                           name="psV")
                            xn = xnT8 if FP8_V else xnTb
                            if FP8_V:
                                for j in range(NE // 2):
                                    nc.tensor.matmul(
                                        psV, xn[:, 2 * j:2 * j + 2, lsl],
                                        wv_sb[:, 2 * j:2 * j + 2, vsl],
                                        start=(j == 0),
                                        stop=(j == NE // 2 - 1
                                              and "v" not in pbr),
                                        perf_mode=W8)
                            else:
                                for j in range(NE):
                                    nc.tensor.matmul(
                                        psV, xn[:, j, lsl],
                                        wv_sb[:, j, vsl],
                                        start=(j == 0),
                                        stop=(j == NE - 1
                                              and "v" not in pbr))
                            if "v" in pbr:
                                nc.tensor.matmul(psV, ones_row[:, 0:PT],
                                                 pbr["v"][:, vsl],
                                                 start=False, stop=True)
                            dst = v3[:, lt, t * 8:(t + 1) * 8, 0:64]
                            src = psV.rearrange("p (h d) -> p h d", h=8)
                            eng = nc.vector if lt % 2 == 0 else nc.scalar
                            if v_dsc is None:
                                eng.tensor_copy(dst, src) \
                                    if eng is nc.vector else eng.copy(dst, src)
                            else:
                                if eng is nc.vector:
                                    eng.tensor_scalar_mul(dst, src, v_dsc)
                                else:
                                    eng.activation(
                                        dst, src,
                                        mybir.ActivationFunctionType.Copy,
                                        scale=v_dsc)

                def emit_norm(p):
                    avA, avB = av_tiles.pop(p)
                    osc = float(WSCALE) if FP8_OUT else 1.0
                    if p == 0 and "avA0" in dump:
                        avd = rp.tile([VW, QL], F32, tag="avd", bufs=1)
                        nc.vector.tensor_copy(avd, avA)
                        dump_tile("avA0", avd[:, :])
                    for hi, av in enumerate((avA, avB)):
                        rr = rp.tile([65, QL], F32, tag=f"rr{hi}", bufs=1)
                        nc.vector.tensor_copy(rr[64:65, :], av[64:65, :])
                        rr0 = rp.tile([1, QL], F32, tag=f"rr0{hi}", bufs=1)
                        nc.sync.dma_start(rr0, rr[64:65, :])
                        rsb = rp.tile([64, QL], F32, tag=f"rsb{hi}", bufs=1)
                        nc.gpsimd.partition_broadcast(rsb, rr0[0:1, :])
                        nc.vector.reciprocal_approx_fast(out=rsb, in_=rsb)
                        if use_mask:
                            nc.vector.tensor_mul(rsb, rsb, mqb)
                        if hi == 0:
                            nc.vector.scalar_tensor_tensor(
                                oT[0:64, p, :], av[0:64, :], osc, rsb,
                                op0=mybir.AluOpType.mult,
                                op1=mybir.AluOpType.mult)
                        else:
                            oto = oop.tile([64, QL], OT_DT, tag="oo")
                            nc.vector.scalar_tensor_tensor(
                                oto, av[0:64, :], osc, rsb,
                                op0=mybir.AluOpType.mult,
                                op1=mybir.AluOpType.mult)
                            nc.sync.dma_start(oT[64:PT, p, :], oto)

                def emit_attn(t):
                    kA, kB, qT = kq_tiles.pop(t)
                    hA, hB = 2 * t, 2 * t + 1
                    avA = avp.tile([VW, QL], F32, tag="avA")
                    avB = avp.tile([VW, QL], F32, tag="avB")
                    av_tiles[t] = (avA, avB)
                    at2 = None
                    for c in range(NL):
                        csl = slice(c * PT, (c + 1) * PT)
                        mm_add = c in MM_ADD
                        bt = bp.tile([PT, 2, QL], FP8 if FP8_BIAS else BF16,
                                     tag="bt")
                        nc.sync.dma_start(bt, bias_d[csl, hA:hB + 1, :])
                        if use_mask:
                            kmb = km_sb[:, c:c + 1]
                        elif FP8_AV:
                            kmb = shift_t
                        else:
                            kmb = 0.0
                        ci = c % 2
                        if ci == 0:
                            at2 = ap.tile([PT, 2, 2, QL], AT_DT, tag="at",
                                          bufs=4)
                        # separate per-head psum tiles + per-head EXPs keep
                        # the A-half pipeline independent of the B-half
                        for hi, kT in enumerate((kA, kB)):
                            psh = psp.tile([PT, QL], F32, tag=f"ps{hi}",
                                           name="psh")
                            nc.tensor.matmul(psh, kT[:, csl], qT,
                                             start=True, stop=not mm_add)
                            if mm_add:
                                nc.tensor.matmul(psh, identb, bt[:, hi, :],
                                                 start=False, stop=True)
                                s_in = psh
                            else:
                                s1 = s1p.tile([PT, QL], BF16, tag=f"s1{hi}")
                                nc.vector.tensor_add(s1, psh, bt[:, hi, :])
                                s_in = s1
                            nc.scalar.activation(
                                at2[:, ci, hi, :], s_in,
                                mybir.ActivationFunctionType.Exp,
                                bias=kmb, scale=gates[hA + hi])
                        if FP8_AV:
                            if ci == 1:
                                cp2 = c // 2
                                for hi, av in enumerate((avA, avB)):
                                    nc.tensor.matmul(
                                        av,
                                        v3[:, c - 1:c + 1, hA + hi, :],
                                        at2[:, :, hi, :],
                                        start=(cp2 == 0),
                                        stop=(cp2 == NL // 2 - 1),
                                        perf_mode=W8)
                        else:
                            for hi, av in enumerate((avA, avB)):
                                nc.tensor.matmul(
                                    av, v3[:, c, hA + hi, :],
                                    at2[:, ci, hi, :],
                                    start=(c == 0), stop=(c == NL - 1))
                        if t == 0 and c <= 1:
                            dump_tile(f"at0{c}", at2[:, ci, :, :])
                        if c == NORM_AT and t - 1 in av_tiles:
                            emit_norm(t - 1)

                emit_proj(0)
                for t in range(1, HP):
                    emit_attn(t - 1)
                    emit_proj(t)
                emit_attn(HP - 1)
                emit_norm(HP - 1)
                dump_tile("v3", v3[:, :, :, :])
                dump_tile("oT", oT[:, :, :])

                xres_sb = yop.tile([PT, 4, L], F32, tag="xres", bufs=1)
                for qb in range(4):
                    nc.sync.dma_start(xres_sb[:, qb, :], xres_d[:, qb, :])

                # ========== out-projection + residual ==========
                for qb in range(4):
                    qsl = slice(qb * PT, (qb + 1) * PT)
                    xr_sb = xres_sb[:, qb, :]
                    y_sb = yop.tile([PT, E], F32, tag="y")
                    for eh in range(2):
                        esl = slice(eh * QL, (eh + 1) * QL)
                        if VW == PT:
                            psF = avp.tile([VW, QL], F32, name="psF",
                                           tag="avA" if (2 * qb + eh) % 2 == 0
                                           else "avB")
                        else:
                            psF = psp.tile([PT, QL], F32, tag="ps0",
                                           name="psF")
                        if FP8_OUT:
                            for m in range(NE // 2):
                                nc.tensor.matmul(
                                    psF, oT[:, 2 * m:2 * m + 2, qsl],
                                    wo_sb[:, 2 * m:2 * m + 2, esl],
                                    start=(m == 0),
                                    stop=(m == NE // 2 - 1
                                          and "o" not in pbr),
                                    perf_mode=W8)
                        else:
                            for m in range(NE):
                                nc.tensor.matmul(
                                    psF, oT[:, m, qsl], wo_sb[:, m, esl],
                                    start=(m == 0),
                                    stop=(m == NE - 1 and "o" not in pbr))
                        if "o" in pbr:
                            nc.tensor.matmul(psF, ones_row[0:1, 0:1],
                                             pbr["o"][:, esl],
                                             start=False, stop=True)
                        if FP8_OUT:
                            nc.vector.scalar_tensor_tensor(
                                y_sb[:, esl], psF, 1.0 / (WSCALE * WSCALE),
                                xr_sb[:, esl],
                                op0=mybir.AluOpType.mult,
                                op1=mybir.AluOpType.add)
                        else:
                            nc.vector.tensor_add(y_sb[:, esl], psF,
                                                 xr_sb[:, esl])
                    nc.sync.dma_start(y_d[qsl, :], y_sb)
    return nc


def _prep_inputs(x, bias, mask, wq, bq, wk, bk, wv, bv, wo, bo, gate,
                 ln_g, ln_b):
    gate = np.asarray(gate, np.float32)
    ln_g = np.asarray(ln_g, np.float32)
    ln_b = np.asarray(ln_b, np.float32)
    grep = np.repeat(gate, D)
    safe_gate = bool(np.all(np.abs(gate) > 1e-6))
    if safe_gate:
        qscale = (SCALE / grep).astype(np.float32)
        exp_scales = [float(g) for g in gate]
    else:
        qscale = np.full(E, SCALE, np.float32)
        exp_scales = [1.0] * H

    sq = WSCALE if FP8_QKPROJ else 1.0
    sv = WSCALE if FP8_V else 1.0
    so = WSCALE if FP8_OUT else 1.0
    wqt = (np.asarray(wq).T * ln_g[:, None] * qscale[None, :] * sq)
    wkt = (np.asarray(wk).T * ln_g[:, None] * sq)
    wvt = (np.asarray(wv).T * ln_g[:, None] * sv)
    wot = (np.asarray(wo).T * so)
    bqe = ((np.asarray(wq) @ ln_b + np.asarray(bq)) * qscale * sq)
    bke = ((np.asarray(wk) @ ln_b + np.asarray(bk)) * sq)
    bve = ((np.asarray(wv) @ ln_b + np.asarray(bv)) * sv)
    # out-proj psum is divided by WSCALE^2 when FP8_OUT (oT and wo both
    # carry WSCALE); residual x is pre-multiplied to compensate.
    boe = np.asarray(bo, np.float32) * (so * so if FP8_OUT else 1.0)
    use_pbias = tuple(bool(np.any(b)) for b in (bqe, bke, bve, boe))

    mask = np.asarray(mask, np.int32)
    use_mask = not bool(np.all(mask == 1))

    def wfmt(w, f8):
        w = np.ascontiguousarray(w.reshape(NE, PT, E).transpose(1, 0, 2))
        return w.astype(FP8_NP if f8 else BF_NP)

    shared = {"wqt": wfmt(wqt, FP8_QKPROJ), "wkt": wfmt(wkt, FP8_QKPROJ),
              "wvt": wfmt(wvt, FP8_V), "wot": wfmt(wot, FP8_OUT)}
    for name, use, b in zip("qkvo", use_pbias, (bqe, bke, bve, boe)):
        if use:
            shared[f"b{name}e"] = b.reshape(1, E).astype(BF_NP)

    x = np.asarray(x, np.float32)
    bias = np.asarray(bias, np.float32)
    in_maps = []
    for c in range(NCORES):
        b_idx, qh = divmod(c, 2)
        q0 = qh * QL
        xr = np.roll(x[b_idx], -q0, axis=0)
        m = {}
        m.update(shared)
        m["xcb"] = np.ascontiguousarray(
            xr.reshape(NL, PT, L).transpose(1, 0, 2)).astype(BF_NP)
        m["xtc"] = np.ascontiguousarray(
            xr.T.reshape(NE, PT, L).transpose(1, 0, 2)).astype(BF_NP)
        m["xres"] = np.ascontiguousarray(
            xr[:QL].reshape(4, PT, L).transpose(1, 0, 2)).astype(np.float32)
        bs = bias[b_idx][:, q0:q0 + QL, :]      # [H, QL, L]
        bs = np.roll(bs, -q0, axis=2)           # roll key axis
        if not safe_gate:
            bs = bs * gate[:, None, None]
        bs = np.ascontiguousarray(bs.transpose(2, 0, 1))  # [L, H, QL]
        m["biasc"] = bs.astype(FP8_NP if FP8_BIAS else BF_NP)
        if use_mask:
            mr = np.roll(mask[b_idx], -q0)
            kmf = (-10000.0 * (1.0 - mr.astype(np.float32))) + SHIFT
            m["kmc"] = np.ascontiguousarray(
                kmf.reshape(NL, PT).T).astype(np.float32)
            m["mqc"] = mr[:QL].astype(np.float32).reshape(1, QL)
        in_maps.append(m)
    return in_maps, (exp_scales, use_pbias, use_mask)


def kernel(**inputs):
    global LAST_RESULT
    in_maps, (exp_scales, use_pbias, use_mask) = _prep_inputs(**inputs)
    nc = _build_nc(exp_scales, use_pbias, use_mask)
    if not nc.is_finalized():
        nc.finalize()
    res = run_bass_kernel_spmd(nc, in_maps, core_ids=list(range(NCORES)))
    LAST_RESULT = res
    out = np.empty((B, L, E), np.float32)
    for c in range(NCORES):
        b_idx, qh = divmod(c, 2)
        out[b_idx, qh * QL:(qh + 1) * QL, :] = res.results[c]["yc"]
    return out
